# revision 24
# baseline (speedup 1.0000x reference)
"""CFConv (SchNet continuous-filter convolution) Trainium2 kernel, v3.

Math (per molecule b):
    rbf[b,i,j,r] = exp(-gamma * (dist[b,i,j] - r*res)^2),  r = 0..299
    f = softplus(rbf @ W1 + b1); f = softplus(f @ W2 + b2)
    out[b,j,c] = sum_i h[b,i,c] * f[b,i,j,c]

Reformulation 1: the filter f[e, c] is a smooth scalar function G_c(d_e)
of the single distance, refit on host onto a Gaussian basis with
exactly-representable bf16 exponents:
    G_c(d) ~= sum_r exp(-g2*(d - 0.1*k_r)^2) * C[r, c]
R2 = 32 centers (0.1*{0,2,..,12}, then 0.4 spacing to 11.0), g2 = 2.5:
the exponent is -(2.5d^2)*1 + d*(0.5k_r) + (-0.025 k_r^2); 0.5*k_r is
exact bf16, 2.5d^2 / d are 3-way bf16-split on host.  C is fit against
the device-simulated basis with error-feedback rounding; end-to-end max
rel err ~5e-3 (gate 2e-2).

Reformulation 2: the h-multiply and i-reduction fold into the second
matmul: out[b,j,c] = sum_{i,r} phi_r(d_bij) * (C[r,c]*h[b,i,c]) runs as
64 PSUM-accumulating matmuls (8 per 512-elem tile) with per-(b, i-set)
stationary weights W[(q,r), c] = C[r,c]*h[b,i_q,c] built on host (bf16).
No DVE/Pool multiply+reduce work at all.

PE micro-scheduling (v3):
  - exp-mms are split into two [K=24, 256] halves on row-quadrants q0
    (rows 0:24) and q32 (rows 32:56); adjacent instructions on distinct
    row groups execute concurrently on the PE.
  - w-mms alternate output column-groups h0/h1: even m accumulate into
    out PSUM at partitions 0:64, odd m at partitions 64:128 (separate
    accumulation chains); host sums the two halves.  Distinct col
    groups can overlap in the array.
  - DMA is spread over the 3 available queues (sync, act, gpsimd) in
    fine pieces ordered by first use, so tile 0 weights and dd arrive
    just after the queue-start floor (~9.2us) and nothing later gates.
Sharding: data-parallel over mb across 8 cores, no collectives.
"""

import numpy as np

MB, ATOM, HD = 32, 64, 64
NCORES = 8
MBC = MB // NCORES            # molecules per core
E = MBC * ATOM * ATOM         # elements per core (b, i, j) order
CH = 512                      # elements per chunk
NT = E // (4 * CH)            # tiles of 4 chunks -> 8
G2 = 2.5
CIDX = np.array(sorted(set(list(range(0, 13, 2)) + list(range(14, 111, 4)))))
R2 = len(CIDX)                # 32
COLSPLIT = False              # w-mms alternate output col-groups h0/h1

_CACHE = {}


def build_bass():
    from contextlib import ExitStack

    import concourse.bass as bass
    from concourse import mybir

    f32 = mybir.dt.float32
    bf16 = mybir.dt.bfloat16
    AF = mybir.ActivationFunctionType

    nc = bass.Bass()
    dd = nc.declare_dram_parameter("dd", [64, 2048], bf16, isOutput=False)
    coef = nc.declare_dram_parameter("coef", [64, 128], bf16, isOutput=False)
    bexp = nc.declare_dram_parameter("bexp", [128, 1], f32, isOutput=False)
    wt = nc.declare_dram_parameter("wt", [128, 2048], bf16, isOutput=False)
    wt45 = nc.declare_dram_parameter("wt45", [128, 1024], bf16, isOutput=False)
    wt67 = nc.declare_dram_parameter("wt67", [128, 1024], bf16, isOutput=False)
    res = nc.declare_dram_parameter("res", [64, 256], f32, isOutput=True)

    with ExitStack() as ctx:
        en = ctx.enter_context

        dd_sb = en(nc.sbuf_tensor("dd_sb", [64, 2048], bf16))
        coef_sb = en(nc.sbuf_tensor("coef_sb", [64, 128], bf16))
        bexp_sb = en(nc.sbuf_tensor("bexp_sb", [128, 1], f32))
        wt_sb = en(nc.sbuf_tensor("wt_sb", [128, 2048], bf16))
        wt45_sb = en(nc.sbuf_tensor("wt45_sb", [128, 1024], bf16))
        wt67_sb = en(nc.sbuf_tensor("wt67_sb", [128, 1024], bf16))
        res_sb = en(nc.sbuf_tensor("res_sb", [64, 256], f32))
        rbf_sb = [en(nc.sbuf_tensor(f"rbf{i}", [128, CH], bf16)) for i in (0, 1)]
        scr_sb = en(nc.sbuf_tensor("scr_sb", [128, 1], f32))

        exp_ps = [en(nc.psum_tensor(f"expps{i}", [128, CH], f32)) for i in (0, 1)]
        outp1 = en(nc.psum_tensor("outp1", [64, 256], f32))
        outp2 = en(nc.psum_tensor("outp2", [128, 256], f32))

        dmaq_s = en(nc.semaphore("dmaq_s"))  # sync: dd, wt t2-t3, stores
        dmaq_a = en(nc.semaphore("dmaq_a"))  # act: coef, bexp, wt t0-t1
        dmaq_g = en(nc.semaphore("dmaq_g"))  # gpsimd: wt t4-t7
        pe_sem = en(nc.semaphore("pe_sem"))
        act_sem = en(nc.semaphore("act_sem"))
        dve_sem = en(nc.semaphore("dve_sem"))

        # PE op counts: eA0,eB0,eA1,eB1, then per t: w(t,0..7), eA/B(t+2)
        PEC = {}
        cnt = 0
        for name in ["e0", "e1"]:
            cnt += 1
            PEC[name] = cnt
        for t in range(NT):
            for m in range(8):
                cnt += 1
                PEC[f"w{t}_{m}"] = cnt
            if t + 2 < NT:
                cnt += 1
                PEC[f"e{t + 2}"] = cnt
        ACTC = {}
        for g in range(NT):
            ACTC[f"x{g}"] = g + 1

        with nc.Block() as block:

            @block.sync
            def _(sy):
                sy.dma_start(dd_sb[:, 0:512], dd[:, 0:512]).then_inc(dmaq_s, 16)
                sy.dma_start(dd_sb[:, 512:1024], dd[:, 512:1024]).then_inc(dmaq_s, 16)
                sy.dma_start(dd_sb[:, 1024:2048], dd[:, 1024:2048]).then_inc(dmaq_s, 16)
                sy.dma_start(wt_sb[:, 1024:1536], wt[:, 1024:1536]).then_inc(dmaq_s, 16)
                sy.dma_start(wt_sb[:, 1536:2048], wt[:, 1536:2048]).then_inc(dmaq_s, 16)
                sy.wait_ge(dve_sem, 2)
                sy.dma_start(res[:, 0:128], res_sb[:, 0:128]).then_inc(dmaq_s, 16)
                sy.wait_ge(dve_sem, 4)
                sy.dma_start(res[:, 128:256], res_sb[:, 128:256]).then_inc(dmaq_s, 16)
                # block-exit drain flushes the in-flight stores

            @block.tensor
            def _(pe):
                def emit_e(g):
                    T = g % 2
                    pe.matmul(
                        exp_ps[g % 2][:],
                        coef_sb[32 * T : 32 * T + 24, :],
                        dd_sb[32 * T : 32 * T + 24, 512 * (g // 2) : 512 * (g // 2) + 512],
                        start=True, stop=True,
                    ).then_inc(pe_sem, 1)

                def emit_w(t, m):
                    b = t // 2
                    if t < 4:
                        wsrc, wcol = wt_sb, 64 * (8 * t + m)
                    elif t < 6:
                        wsrc, wcol = wt45_sb, 64 * (8 * (t - 4) + m)
                    else:
                        wsrc, wcol = wt67_sb, 64 * (8 * (t - 6) + m)
                    if COLSPLIT:
                        ps = (outp1[:, 64 * b : 64 * b + 64] if m % 2 == 0
                              else outp2[64:128, 64 * b : 64 * b + 64])
                        start = (t % 2 == 0 and m < 2)
                        stop = (t % 2 == 1 and m >= 6)
                    else:
                        ps = outp1[:, 64 * b : 64 * b + 64]
                        start = (t % 2 == 0 and m == 0)
                        stop = (t % 2 == 1 and m == 7)
                    pe.matmul(
                        ps,
                        wsrc[:, wcol : wcol + 64],
                        rbf_sb[t % 2][:, 64 * m : 64 * m + 64],
                        start=start, stop=stop,
                    ).then_inc(pe_sem, 1)

                pe.wait_ge(dmaq_a, 16)       # coef
                pe.wait_ge(dmaq_s, 16)       # dd piece 0a (tiles 0-1)
                emit_e(0)
                emit_e(1)
                for t in range(NT):
                    if t == 0:
                        pe.wait_ge(dmaq_a, 48)   # wt tile 0
                    elif t == 1:
                        pe.wait_ge(dmaq_a, 64)   # wt tile 1
                    elif t == 2:
                        pe.wait_ge(dmaq_s, 64)   # wt tile 2
                    elif t == 3:
                        pe.wait_ge(dmaq_s, 80)   # wt tile 3
                    elif t == 4:
                        pe.wait_ge(dmaq_g, 16)   # wt tiles 4-5
                    elif t == 6:
                        pe.wait_ge(dmaq_g, 32)   # wt tiles 6-7
                    pe.wait_ge(act_sem, ACTC[f"x{t}"])
                    for m in range(8):
                        emit_w(t, m)
                    if t + 2 < NT:
                        if t + 2 == 2:
                            pe.wait_ge(dmaq_s, 32)   # dd piece 0b
                        elif t + 2 == 4:
                            pe.wait_ge(dmaq_s, 48)   # dd piece 1
                        emit_e(t + 2)

            @block.scalar
            def _(act):
                act.dma_start(coef_sb[:], coef[:]).then_inc(dmaq_a, 16)
                act.dma_start(bexp_sb[:], bexp[:]).then_inc(dmaq_a, 16)
                act.dma_start(wt_sb[:, 0:512], wt[:, 0:512]).then_inc(dmaq_a, 16)
                act.dma_start(wt_sb[:, 512:1024], wt[:, 512:1024]).then_inc(dmaq_a, 16)
                # dummy op pulls in the Exp table while DMAs fly
                act.activation(scr_sb[:], scr_sb[:], AF.Exp, bias=0.0)
                act.wait_ge(dmaq_a, 32)      # bexp
                for g in range(0, NT):
                    act.wait_ge(pe_sem, PEC[f"e{g}"])
                    # rbf_sb[g%2] WAR vs w(g-2): subsumed (eB(g) follows
                    # w(g-2) in PE order)
                    act.activation(
                        rbf_sb[g % 2][:], exp_ps[g % 2][:], AF.Exp,
                        bias=bexp_sb[:],
                    ).then_inc(act_sem, 1)

            @block.vector
            def _(ve):
                for b in range(MBC):
                    ve.wait_ge(pe_sem, PEC[f"w{2 * b + 1}_7"])
                    ve.tensor_scalar_mul(
                        res_sb[:, 64 * b : 64 * b + 64],
                        outp1[:, 64 * b : 64 * b + 64], 1.0
                    ).then_inc(dve_sem, 1)

            @block.gpsimd
            def _(po):
                po.dma_start(wt45_sb[:], wt45[:]).then_inc(dmaq_g, 16)
                po.dma_start(wt67_sb[:], wt67[:]).then_inc(dmaq_g, 16)

    return nc


def _split_bf(x, n):
    """Split fp32 array into n bf16 components summing to ~x."""
    import ml_dtypes

    bf = ml_dtypes.bfloat16
    x = x.astype(np.float32)
    parts = []
    for _ in range(n):
        p = x.astype(bf)
        parts.append(p)
        x = x - p.astype(np.float32)
    return parts


def _fit_filter(W1, b1, W2, b2):
    """Refit the 2-layer filter MLP as an R2-term gaussian expansion.

    Returns C [R2, HD] bf16-held-as-f32, fit against the device-simulated
    (bf16-split + bf16-exp) basis with error-feedback rounding.
    """
    import ml_dtypes

    bf = ml_dtypes.bfloat16
    f4 = np.float32
    grid = np.linspace(0, 10, 16001).astype(f4)
    centers300 = np.arange(300) * 0.1
    rbfg = np.exp(-10.0 * (grid[:, None].astype(np.float64) - centers300) ** 2)
    z = rbfg @ W1.astype(np.float64) + b1.astype(np.float64)
    z = np.logaddexp(0, z) @ W2.astype(np.float64) + b2.astype(np.float64)
    Gt = np.logaddexp(0, z)

    s_parts = _split_bf(np.float32(G2) * grid * grid, 3)
    t_parts = _split_bf(grid, 3)
    cc = (0.1 * CIDX).astype(np.float64)
    kco = (np.float32(2 * G2 * 0.1) * CIDX.astype(f4)).astype(f4)  # 0.5*k
    bias = (np.float32(-G2) * (cc.astype(f4) ** 2)).astype(f4)
    zd = (
        -sum(p[:, None].astype(np.float64) for p in s_parts)
        + sum(p[:, None].astype(np.float64) for p in t_parts)
        * kco.astype(bf).astype(np.float64)
        + bias.astype(np.float64)
    )
    Ad = np.exp(zd).astype(f4).astype(bf).astype(np.float64)

    lam = 1e-7 * len(grid) / R2
    M = Ad.T @ Ad + lam * np.eye(R2)
    C = np.linalg.solve(M, Ad.T @ Gt)
    for _ in range(6):
        Cq = C.astype(f4).astype(bf).astype(np.float64)
        C = Cq + np.linalg.solve(M, Ad.T @ (Gt - Ad @ Cq))
    return C.astype(f4).astype(bf).astype(f4)


def host_prep(h, dist, W1, b1, W2, b2):
    """Build per-core input maps (weight-sized fit + layout prep)."""
    import ml_dtypes

    bf = ml_dtypes.bfloat16
    f4 = np.float32

    wkey = (W1.tobytes(), b1.tobytes(), W2.tobytes(), b2.tobytes())
    ckey = hash(wkey)
    if _CACHE.get("ckey") != ckey:
        _CACHE["C"] = _fit_filter(W1, b1, W2, b2)
        _CACHE["ckey"] = ckey
    C = _CACHE["C"]  # [R2, 64] f32 (bf16 values)

    kco = (np.float32(2 * G2 * 0.1) * CIDX.astype(f4)).astype(f4)
    coefm = np.zeros((64, 128), f4)
    for T in range(2):
        for q in range(4):
            coefm[32 * T + 6 * q + 0 : 32 * T + 6 * q + 3, 32 * q : 32 * q + 32] = -1.0
            coefm[32 * T + 6 * q + 3 : 32 * T + 6 * q + 6, 32 * q : 32 * q + 32] = kco
    coefm = np.ascontiguousarray(coefm.astype(bf))

    cc = (0.1 * CIDX).astype(np.float64)
    bias = (np.float32(-G2) * (cc.astype(f4) ** 2)).astype(f4)
    bexpm = np.zeros((128, 1), f4)
    for q in range(4):
        bexpm[32 * q : 32 * q + 32, 0] = bias
    bexpm = np.ascontiguousarray(bexpm)

    in_maps = []
    for gcore in range(NCORES):
        dist_c = dist[gcore * MBC : (gcore + 1) * MBC].astype(f4)
        d = dist_c.reshape(-1)                        # (b, i, j) order
        s3 = np.stack(_split_bf(np.float32(G2) * d * d, 3))  # [3, E]
        t3 = np.stack(_split_bf(d, 3))
        ddm = np.zeros((64, 2048), bf)
        for t in range(NT):
            for q in range(4):
                ch = 4 * t + q
                br, bc = 32 * (t % 2), 512 * (t // 2)
                ddm[br + 6 * q + 0 : br + 6 * q + 3, bc : bc + 512] = \
                    s3[:, ch * 512 : ch * 512 + 512]
                ddm[br + 6 * q + 3 : br + 6 * q + 6, bc : bc + 512] = \
                    t3[:, ch * 512 : ch * 512 + 512]

        h_c = h[gcore * MBC : (gcore + 1) * MBC].astype(f4)   # [4, 64, 64]
        # W[b, i, r, c] = bf16(C[r, c] * h[b, i, c])
        Wf = (C[None, None] * h_c[:, :, None, :]).astype(bf)  # [4, 64, 32, 64]
        wtm = np.zeros((128, 4096), bf)
        for t in range(NT):
            b = t // 2
            for m in range(8):
                col = 64 * (8 * t + m)
                for q in range(4):
                    i = (4 * (t % 2) + q) * 8 + m
                    wtm[32 * q : 32 * q + 32, col : col + 64] = Wf[b, i]

        in_maps.append(
            {"dd": np.ascontiguousarray(ddm), "coef": coefm, "bexp": bexpm,
             "wt": np.ascontiguousarray(wtm[:, 0:2048]),
             "wt45": np.ascontiguousarray(wtm[:, 2048:3072]),
             "wt67": np.ascontiguousarray(wtm[:, 3072:4096])}
        )
    return in_maps


def decode_res(res_np):
    """res [128, 256] -> out_core [MBC, ATOM(j), HD(c)].

    out[b, j, c] = res[c, 64b+j] + res[64+c, 64b+j] (the two col-group
    accumulation halves)."""
    return np.ascontiguousarray(
        res_np.reshape(HD, MBC, ATOM).transpose(1, 2, 0)
    )


def kernel(h, dist, W1, b1, W2, b2):
    from concourse.bass_utils import run_bass_kernel_spmd

    if "nc" not in _CACHE:
        _CACHE["nc"] = build_bass()
    nc = _CACHE["nc"]
    in_maps = host_prep(h, dist, W1, b1, W2, b2)
    out = run_bass_kernel_spmd(nc, in_maps, list(range(NCORES)))
    cores = [decode_res(out.results[g]["res"]) for g in range(NCORES)]
    return np.concatenate(cores, axis=0).astype(np.float32)


# revision 26
# speedup vs baseline: 1.0358x; 1.0358x over previous
"""CFConv (SchNet continuous-filter convolution) Trainium2 kernel, v3.

Math (per molecule b):
    rbf[b,i,j,r] = exp(-gamma * (dist[b,i,j] - r*res)^2),  r = 0..299
    f = softplus(rbf @ W1 + b1); f = softplus(f @ W2 + b2)
    out[b,j,c] = sum_i h[b,i,c] * f[b,i,j,c]

Reformulation 1: the filter f[e, c] is a smooth scalar function G_c(d_e)
of the single distance, refit on host onto a Gaussian basis with
exactly-representable bf16 exponents:
    G_c(d) ~= sum_r exp(-g2*(d - 0.1*k_r)^2) * C[r, c]
R2 = 32 centers (0.1*{0,2,..,12}, then 0.4 spacing to 11.0), g2 = 2.5:
the exponent is -(2.5d^2)*1 + d*(0.5k_r) + (-0.025 k_r^2); 0.5*k_r is
exact bf16, 2.5d^2 / d are 3-way bf16-split on host.  C is fit against
the device-simulated basis with error-feedback rounding; end-to-end max
rel err ~5e-3 (gate 2e-2).

Reformulation 2: the h-multiply and i-reduction fold into the second
matmul: out[b,j,c] = sum_{i,r} phi_r(d_bij) * (C[r,c]*h[b,i,c]) runs as
64 PSUM-accumulating matmuls (8 per 512-elem tile) with per-(b, i-set)
stationary weights W[(q,r), c] = C[r,c]*h[b,i_q,c] built on host (bf16).
No DVE/Pool multiply+reduce work at all.

PE micro-scheduling (v3):
  - exp-mms are split into two [K=24, 256] halves on row-quadrants q0
    (rows 0:24) and q32 (rows 32:56); adjacent instructions on distinct
    row groups execute concurrently on the PE.
  - w-mms alternate output column-groups h0/h1: even m accumulate into
    out PSUM at partitions 0:64, odd m at partitions 64:128 (separate
    accumulation chains); host sums the two halves.  Distinct col
    groups can overlap in the array.
  - DMA is spread over the 3 available queues (sync, act, gpsimd) in
    fine pieces ordered by first use, so tile 0 weights and dd arrive
    just after the queue-start floor (~9.2us) and nothing later gates.
Sharding: data-parallel over mb across 8 cores, no collectives.
"""

import numpy as np

MB, ATOM, HD = 32, 64, 64
NCORES = 8
MBC = MB // NCORES            # molecules per core
E = MBC * ATOM * ATOM         # elements per core (b, i, j) order
CH = 512                      # elements per chunk
NT = E // (4 * CH)            # tiles of 4 chunks -> 8
G2 = 2.5
CIDX = np.array(sorted(set(list(range(0, 13, 2)) + list(range(14, 111, 4)))))
R2 = len(CIDX)                # 32
COLSPLIT = False              # w-mms alternate output col-groups h0/h1

_CACHE = {}


def build_bass():
    from contextlib import ExitStack

    import concourse.bass as bass
    from concourse import mybir

    f32 = mybir.dt.float32
    bf16 = mybir.dt.bfloat16
    AF = mybir.ActivationFunctionType

    nc = bass.Bass()
    dd = nc.declare_dram_parameter("dd", [64, 2048], bf16, isOutput=False)
    coef = nc.declare_dram_parameter("coef", [64, 128], bf16, isOutput=False)
    bexp = nc.declare_dram_parameter("bexp", [128, 1], f32, isOutput=False)
    wt = nc.declare_dram_parameter("wt", [128, 2048], bf16, isOutput=False)
    wt45 = nc.declare_dram_parameter("wt45", [128, 1024], bf16, isOutput=False)
    wt67 = nc.declare_dram_parameter("wt67", [128, 1024], bf16, isOutput=False)
    res = nc.declare_dram_parameter("res", [64, 256], f32, isOutput=True)

    with ExitStack() as ctx:
        en = ctx.enter_context

        dd_sb = en(nc.sbuf_tensor("dd_sb", [64, 2048], bf16))
        coef_sb = en(nc.sbuf_tensor("coef_sb", [64, 128], bf16))
        bexp_sb = en(nc.sbuf_tensor("bexp_sb", [128, 1], f32))
        wt_sb = en(nc.sbuf_tensor("wt_sb", [128, 2048], bf16))
        wt45_sb = en(nc.sbuf_tensor("wt45_sb", [128, 1024], bf16))
        wt67_sb = en(nc.sbuf_tensor("wt67_sb", [128, 1024], bf16))
        res_sb = en(nc.sbuf_tensor("res_sb", [64, 256], f32))
        rbf_sb = [en(nc.sbuf_tensor(f"rbf{i}", [128, CH], bf16)) for i in (0, 1)]
        scr_sb = en(nc.sbuf_tensor("scr_sb", [128, 1], f32))

        exp_ps = [en(nc.psum_tensor(f"expps{i}", [128, CH], f32)) for i in (0, 1)]
        outp1 = en(nc.psum_tensor("outp1", [64, 256], f32))
        outp2 = en(nc.psum_tensor("outp2", [128, 256], f32))

        dmaq_s = en(nc.semaphore("dmaq_s"))  # sync: dd, wt t2-t3, stores
        dmaq_a = en(nc.semaphore("dmaq_a"))  # act: coef, bexp, wt t0-t1
        dmaq_g = en(nc.semaphore("dmaq_g"))  # gpsimd: wt t4-t7
        pe_sem = en(nc.semaphore("pe_sem"))
        act_sem = en(nc.semaphore("act_sem"))
        dve_sem = en(nc.semaphore("dve_sem"))

        # PE op counts: eA0,eB0,eA1,eB1, then per t: w(t,0..7), eA/B(t+2)
        PEC = {}
        cnt = 0
        for name in ["e0", "e1"]:
            cnt += 1
            PEC[name] = cnt
        for t in range(NT):
            for m in range(8):
                cnt += 1
                PEC[f"w{t}_{m}"] = cnt
            if t + 2 < NT:
                cnt += 1
                PEC[f"e{t + 2}"] = cnt
        ACTC = {}
        for g in range(NT):
            ACTC[f"x{g}"] = g + 1

        with nc.Block() as block:

            @block.sync
            def _(sy):
                sy.dma_start(dd_sb[:, 0:512], dd[:, 0:512]).then_inc(dmaq_s, 16)
                sy.dma_start(dd_sb[:, 512:1024], dd[:, 512:1024]).then_inc(dmaq_s, 16)
                # DMA fabric bandwidth is shared across queues: hold the
                # non-critical pieces back until the w0-gating wt tile 0
                # has landed, so the early fabric window serves it alone.
                sy.wait_ge(dmaq_a, 48)
                sy.dma_start(dd_sb[:, 1024:2048], dd[:, 1024:2048]).then_inc(dmaq_s, 16)
                sy.dma_start(wt_sb[:, 1024:1536], wt[:, 1024:1536]).then_inc(dmaq_s, 16)
                sy.dma_start(wt_sb[:, 1536:2048], wt[:, 1536:2048]).then_inc(dmaq_s, 16)
                sy.wait_ge(dve_sem, 2)
                sy.dma_start(res[:, 0:128], res_sb[:, 0:128]).then_inc(dmaq_s, 16)
                sy.wait_ge(dve_sem, 4)
                sy.dma_start(res[:, 128:256], res_sb[:, 128:256]).then_inc(dmaq_s, 16)
                # block-exit drain flushes the in-flight stores

            @block.tensor
            def _(pe):
                def emit_e(g):
                    T = g % 2
                    pe.matmul(
                        exp_ps[g % 2][:],
                        coef_sb[32 * T : 32 * T + 24, :],
                        dd_sb[32 * T : 32 * T + 24, 512 * (g // 2) : 512 * (g // 2) + 512],
                        start=True, stop=True,
                    ).then_inc(pe_sem, 1)

                def emit_w(t, m):
                    b = t // 2
                    if t < 4:
                        wsrc, wcol = wt_sb, 64 * (8 * t + m)
                    elif t < 6:
                        wsrc, wcol = wt45_sb, 64 * (8 * (t - 4) + m)
                    else:
                        wsrc, wcol = wt67_sb, 64 * (8 * (t - 6) + m)
                    if COLSPLIT:
                        ps = (outp1[:, 64 * b : 64 * b + 64] if m % 2 == 0
                              else outp2[64:128, 64 * b : 64 * b + 64])
                        start = (t % 2 == 0 and m < 2)
                        stop = (t % 2 == 1 and m >= 6)
                    else:
                        ps = outp1[:, 64 * b : 64 * b + 64]
                        start = (t % 2 == 0 and m == 0)
                        stop = (t % 2 == 1 and m == 7)
                    pe.matmul(
                        ps,
                        wsrc[:, wcol : wcol + 64],
                        rbf_sb[t % 2][:, 64 * m : 64 * m + 64],
                        start=start, stop=stop,
                    ).then_inc(pe_sem, 1)

                pe.wait_ge(dmaq_a, 16)       # coef
                pe.wait_ge(dmaq_s, 16)       # dd piece 0a (tiles 0-1)
                emit_e(0)
                emit_e(1)
                for t in range(NT):
                    if t == 0:
                        pe.wait_ge(dmaq_a, 48)   # wt tile 0
                    elif t == 1:
                        pe.wait_ge(dmaq_a, 64)   # wt tile 1
                    elif t == 2:
                        pe.wait_ge(dmaq_s, 64)   # wt tile 2
                    elif t == 3:
                        pe.wait_ge(dmaq_s, 80)   # wt tile 3
                    elif t == 4:
                        pe.wait_ge(dmaq_g, 16)   # wt tiles 4-5
                    elif t == 6:
                        pe.wait_ge(dmaq_g, 32)   # wt tiles 6-7
                    pe.wait_ge(act_sem, ACTC[f"x{t}"])
                    for m in range(8):
                        emit_w(t, m)
                    if t + 2 < NT:
                        if t + 2 == 2:
                            pe.wait_ge(dmaq_s, 32)   # dd piece 0b
                        elif t + 2 == 4:
                            pe.wait_ge(dmaq_s, 48)   # dd piece 1
                        emit_e(t + 2)

            @block.scalar
            def _(act):
                act.dma_start(coef_sb[:], coef[:]).then_inc(dmaq_a, 16)
                act.dma_start(bexp_sb[:], bexp[:]).then_inc(dmaq_a, 16)
                act.dma_start(wt_sb[:, 0:512], wt[:, 0:512]).then_inc(dmaq_a, 16)
                act.dma_start(wt_sb[:, 512:1024], wt[:, 512:1024]).then_inc(dmaq_a, 16)
                # dummy op pulls in the Exp table while DMAs fly
                act.activation(scr_sb[:], scr_sb[:], AF.Exp, bias=0.0)
                act.wait_ge(dmaq_a, 32)      # bexp
                for g in range(0, NT):
                    act.wait_ge(pe_sem, PEC[f"e{g}"])
                    # rbf_sb[g%2] WAR vs w(g-2): subsumed (eB(g) follows
                    # w(g-2) in PE order)
                    act.activation(
                        rbf_sb[g % 2][:], exp_ps[g % 2][:], AF.Exp,
                        bias=bexp_sb[:],
                    ).then_inc(act_sem, 1)

            @block.vector
            def _(ve):
                for b in range(MBC):
                    ve.wait_ge(pe_sem, PEC[f"w{2 * b + 1}_7"])
                    ve.tensor_scalar_mul(
                        res_sb[:, 64 * b : 64 * b + 64],
                        outp1[:, 64 * b : 64 * b + 64], 1.0
                    ).then_inc(dve_sem, 1)

            @block.gpsimd
            def _(po):
                # tiles 4-7 are needed last: stay off the fabric until the
                # head-critical act-queue pieces (wt tiles 0-1) are in
                po.wait_ge(dmaq_a, 64)
                po.dma_start(wt45_sb[:], wt45[:]).then_inc(dmaq_g, 16)
                po.dma_start(wt67_sb[:], wt67[:]).then_inc(dmaq_g, 16)

    return nc


def _split_bf(x, n):
    """Split fp32 array into n bf16 components summing to ~x."""
    import ml_dtypes

    bf = ml_dtypes.bfloat16
    x = x.astype(np.float32)
    parts = []
    for _ in range(n):
        p = x.astype(bf)
        parts.append(p)
        x = x - p.astype(np.float32)
    return parts


def _fit_filter(W1, b1, W2, b2):
    """Refit the 2-layer filter MLP as an R2-term gaussian expansion.

    Returns C [R2, HD] bf16-held-as-f32, fit against the device-simulated
    (bf16-split + bf16-exp) basis with error-feedback rounding.
    """
    import ml_dtypes

    bf = ml_dtypes.bfloat16
    f4 = np.float32
    grid = np.linspace(0, 10, 16001).astype(f4)
    centers300 = np.arange(300) * 0.1
    rbfg = np.exp(-10.0 * (grid[:, None].astype(np.float64) - centers300) ** 2)
    z = rbfg @ W1.astype(np.float64) + b1.astype(np.float64)
    z = np.logaddexp(0, z) @ W2.astype(np.float64) + b2.astype(np.float64)
    Gt = np.logaddexp(0, z)

    s_parts = _split_bf(np.float32(G2) * grid * grid, 3)
    t_parts = _split_bf(grid, 3)
    cc = (0.1 * CIDX).astype(np.float64)
    kco = (np.float32(2 * G2 * 0.1) * CIDX.astype(f4)).astype(f4)  # 0.5*k
    bias = (np.float32(-G2) * (cc.astype(f4) ** 2)).astype(f4)
    zd = (
        -sum(p[:, None].astype(np.float64) for p in s_parts)
        + sum(p[:, None].astype(np.float64) for p in t_parts)
        * kco.astype(bf).astype(np.float64)
        + bias.astype(np.float64)
    )
    Ad = np.exp(zd).astype(f4).astype(bf).astype(np.float64)

    lam = 1e-7 * len(grid) / R2
    M = Ad.T @ Ad + lam * np.eye(R2)
    C = np.linalg.solve(M, Ad.T @ Gt)
    for _ in range(6):
        Cq = C.astype(f4).astype(bf).astype(np.float64)
        C = Cq + np.linalg.solve(M, Ad.T @ (Gt - Ad @ Cq))
    return C.astype(f4).astype(bf).astype(f4)


def host_prep(h, dist, W1, b1, W2, b2):
    """Build per-core input maps (weight-sized fit + layout prep)."""
    import ml_dtypes

    bf = ml_dtypes.bfloat16
    f4 = np.float32

    wkey = (W1.tobytes(), b1.tobytes(), W2.tobytes(), b2.tobytes())
    ckey = hash(wkey)
    if _CACHE.get("ckey") != ckey:
        _CACHE["C"] = _fit_filter(W1, b1, W2, b2)
        _CACHE["ckey"] = ckey
    C = _CACHE["C"]  # [R2, 64] f32 (bf16 values)

    kco = (np.float32(2 * G2 * 0.1) * CIDX.astype(f4)).astype(f4)
    coefm = np.zeros((64, 128), f4)
    for T in range(2):
        for q in range(4):
            coefm[32 * T + 6 * q + 0 : 32 * T + 6 * q + 3, 32 * q : 32 * q + 32] = -1.0
            coefm[32 * T + 6 * q + 3 : 32 * T + 6 * q + 6, 32 * q : 32 * q + 32] = kco
    coefm = np.ascontiguousarray(coefm.astype(bf))

    cc = (0.1 * CIDX).astype(np.float64)
    bias = (np.float32(-G2) * (cc.astype(f4) ** 2)).astype(f4)
    bexpm = np.zeros((128, 1), f4)
    for q in range(4):
        bexpm[32 * q : 32 * q + 32, 0] = bias
    bexpm = np.ascontiguousarray(bexpm)

    in_maps = []
    for gcore in range(NCORES):
        dist_c = dist[gcore * MBC : (gcore + 1) * MBC].astype(f4)
        d = dist_c.reshape(-1)                        # (b, i, j) order
        s3 = np.stack(_split_bf(np.float32(G2) * d * d, 3))  # [3, E]
        t3 = np.stack(_split_bf(d, 3))
        ddm = np.zeros((64, 2048), bf)
        for t in range(NT):
            for q in range(4):
                ch = 4 * t + q
                br, bc = 32 * (t % 2), 512 * (t // 2)
                ddm[br + 6 * q + 0 : br + 6 * q + 3, bc : bc + 512] = \
                    s3[:, ch * 512 : ch * 512 + 512]
                ddm[br + 6 * q + 3 : br + 6 * q + 6, bc : bc + 512] = \
                    t3[:, ch * 512 : ch * 512 + 512]

        h_c = h[gcore * MBC : (gcore + 1) * MBC].astype(f4)   # [4, 64, 64]
        # W[b, i, r, c] = bf16(C[r, c] * h[b, i, c])
        Wf = (C[None, None] * h_c[:, :, None, :]).astype(bf)  # [4, 64, 32, 64]
        wtm = np.zeros((128, 4096), bf)
        for t in range(NT):
            b = t // 2
            for m in range(8):
                col = 64 * (8 * t + m)
                for q in range(4):
                    i = (4 * (t % 2) + q) * 8 + m
                    wtm[32 * q : 32 * q + 32, col : col + 64] = Wf[b, i]

        in_maps.append(
            {"dd": np.ascontiguousarray(ddm), "coef": coefm, "bexp": bexpm,
             "wt": np.ascontiguousarray(wtm[:, 0:2048]),
             "wt45": np.ascontiguousarray(wtm[:, 2048:3072]),
             "wt67": np.ascontiguousarray(wtm[:, 3072:4096])}
        )
    return in_maps


def decode_res(res_np):
    """res [128, 256] -> out_core [MBC, ATOM(j), HD(c)].

    out[b, j, c] = res[c, 64b+j] + res[64+c, 64b+j] (the two col-group
    accumulation halves)."""
    return np.ascontiguousarray(
        res_np.reshape(HD, MBC, ATOM).transpose(1, 2, 0)
    )


def kernel(h, dist, W1, b1, W2, b2):
    from concourse.bass_utils import run_bass_kernel_spmd

    if "nc" not in _CACHE:
        _CACHE["nc"] = build_bass()
    nc = _CACHE["nc"]
    in_maps = host_prep(h, dist, W1, b1, W2, b2)
    out = run_bass_kernel_spmd(nc, in_maps, list(range(NCORES)))
    cores = [decode_res(out.results[g]["res"]) for g in range(NCORES)]
    return np.concatenate(cores, axis=0).astype(np.float32)
